# revision 1
# baseline (speedup 1.0000x reference)
"""Multi-head attention with relative-position-bias MLP on 8 TRN2 NeuronCores.

Strategy: pure data-parallel over batch (B=8 -> 1 batch element per core, no
collectives). Host-side prep is layout only: per-core transposed x (plus a
token-reversed copy feeding k/v), transposed weights, replicated proj bias,
and exp() of the 63x63 relative-position bias table (the bias is a
2D-Toeplitz expansion of a tiny MLP on 63*63 distinct (rel_x, rel_y) points;
~7 MFLOP of a 66 GFLOP problem).

Device algorithm per core (N=1024 tokens, C=768, H=12 heads, D=64):
  qT[o,n] = qkv_wT[:, o].T @ xT
  kT[o,n] = qkv_wT[:, o].T @ xRT      (token-reversed k)
  v[n,o]  = xRT.T @ qkv_wT[:, v-sec]  (token-reversed v, + ones column)
  per head pair (2j, 2j+1), k-tile t (128 reversed tokens):
     sT_h = kT_h(t).T @ qT_h          [nk=128, nq=1024] (pair interleaved so
                                       the two K=64 matmuls overlap in PE)
     E = exp(sT/8)                    (ACT, scale folded into exp)
     P = E * expB_tile                (DVE/GPSIMD alternating, all-SBUF;
                                       exp(s+b) = exp(s)*exp(b))
     av[h,c] += [v_h(t) | 1].T @ P    (PE accumulate; row 64 = colsum)
  outT_h(c) = av[0:64] * recip(av[64])  (recip replicated via K=1 f32r MM)
  final = outT.T @ proj_wT (+ proj_b via K=1 MM accumulate)

Token reversal trick: bias[h,n,m] depends on grid coords of (n,m) only via
(cy_n - cy_m, cx_n - cx_m). Reversing key/value token order makes the
Toeplitz expansion all-positive-stride: TBLREP_h[p, J] = expG_h[63*(p//32)
+ p%32 + J] (4 plain DMAs per head), and each [128,1024] bias tile is a
strided view of it. The AV reduction over k-tiles is order-invariant.
"""
import sys

import numpy as np

sys.path.insert(0, "/opt/trn_rl_repo")

import concourse.bass as bass  # noqa: E402
import concourse.mybir as mybir  # noqa: E402
import concourse.tile as tile  # noqa: E402
from concourse import bacc  # noqa: E402
from concourse.bass_utils import run_bass_kernel_spmd  # noqa: E402

F32 = mybir.dt.float32
F32R = mybir.dt.float32r
BF16 = mybir.dt.bfloat16
EXP = mybir.ActivationFunctionType.Exp
COPY = mybir.ActivationFunctionType.Copy

B, N, C, H, D = 8, 1024, 768, 12, 64
SCALE = float(D) ** -0.5
NT = N // 128   # 8 token tiles
CT = C // 128   # 6 channel tiles
TBLW = 3781     # TBLREP width (padded so 2016-wide views stay in range)
TW = 4001       # DRAM table width per head (>= 220 + TBLW, zero-padded)


def _build_graph():
    nc = bacc.Bacc("TRN2", target_bir_lowering=False, debug=False,
                   enable_asserts=False, num_devices=B)
    xT_d = nc.dram_tensor("xT", [C, N], F32, kind="ExternalInput")
    xRT_d = nc.dram_tensor("xRT", [C, N], F32, kind="ExternalInput")
    wqkv_d = nc.dram_tensor("qkv_wT", [C, 3 * C], F32, kind="ExternalInput")
    wproj_d = nc.dram_tensor("proj_wT", [C, C], F32, kind="ExternalInput")
    pbrep_d = nc.dram_tensor("proj_b_rep", [128, C], F32, kind="ExternalInput")
    tbl_d = nc.dram_tensor("rpb_tbl", [H, TW], BF16, kind="ExternalInput")
    out_d = nc.dram_tensor("out", [N, C], F32, kind="ExternalOutput")

    with tile.TileContext(nc) as tc:
        _kern(tc, nc, xT_d, xRT_d, wqkv_d, wproj_d, pbrep_d, tbl_d, out_d)
    nc.compile()
    return nc


def _kern(tc, nc, xT_d, xRT_d, wqkv_d, wproj_d, pbrep_d, tbl_d, out_d):
    from contextlib import ExitStack

    with ExitStack() as es:
        persist = es.enter_context(tc.tile_pool(name="persist", bufs=1))
        # qT tiles 0..5, kT tiles 6..11; [o-part, n-free]
        qk_sb = [persist.tile([128, N], F32R, tag=f"qk{i}", name=f"qk{i}")
                 for i in range(12)]
        # v (token-reversed) head-strided with ones column at h*65+64
        vaug = [persist.tile([128, H * 65], BF16, tag=f"va{i}", name=f"va{i}")
                for i in range(NT)]
        # attention output transposed [c, n], c = h*64+d
        outT = [persist.tile([128, N], F32R, tag=f"ot{i}", name=f"ot{i}")
                for i in range(CT)]
        ones_f = persist.tile([128, 64], F32, tag="onesf")
        nc.vector.memset(ones_f[:], 1.0)
        ones_r = persist.tile([128, 128], F32R, tag="onesr")
        nc.vector.tensor_copy(ones_r[:, 0:64], ones_f[:])
        nc.vector.tensor_copy(ones_r[:, 64:128], ones_f[:])
        onescol = persist.tile([128, H], F32, tag="onescol")
        nc.vector.memset(onescol[:], 1.0)
        for t in range(NT):
            va_v = vaug[t][:].rearrange("p (h e) -> p h e", e=65)
            nc.vector.tensor_copy(va_v[:, :, 64:65], onescol[:].unsqueeze(-1))

        # ---------------- QKV ----------------
        with ExitStack() as esq:
            ld = esq.enter_context(tc.tile_pool(name="ld", bufs=1))
            xT = [ld.tile([128, N], F32R, tag=f"x{i}", name=f"x{i}")
                  for i in range(CT)]
            xRT = [ld.tile([128, N], F32R, tag=f"xr{i}", name=f"xr{i}")
                   for i in range(CT)]
            wq = [ld.tile([128, C], F32R, tag=f"wq{i}", name=f"wq{i}")
                  for i in range(CT)]
            wk = [ld.tile([128, C], F32R, tag=f"wk{i}", name=f"wk{i}")
                  for i in range(CT)]
            for i in range(CT):
                nc.gpsimd.dma_start(xT[i][:], xT_d.ap()[i * 128:(i + 1) * 128, :])
                nc.gpsimd.dma_start(xRT[i][:],
                                    xRT_d.ap()[i * 128:(i + 1) * 128, :])
                nc.gpsimd.dma_start(wq[i][:],
                                    wqkv_d.ap()[i * 128:(i + 1) * 128, 0:C])
                nc.gpsimd.dma_start(wk[i][:],
                                    wqkv_d.ap()[i * 128:(i + 1) * 128, C:2 * C])
            qps = esq.enter_context(tc.tile_pool(name="qps", bufs=4, space="PSUM"))
            for ot in range(12):
                rhs_src = xT if ot < 6 else xRT
                for c in range(2):
                    ps = qps.tile([128, 512], F32, tag="ps", name="qkps", bufs=6)
                    wsrc = wq if ot < 6 else wk
                    oo = (ot % 6) * 128
                    for kt in range(CT):
                        nc.tensor.matmul(
                            ps[:], wsrc[kt][:, oo:oo + 128],
                            rhs_src[kt][:, c * 512:(c + 1) * 512],
                            start=(kt == 0), stop=(kt == CT - 1))
                    nc.vector.tensor_copy(qk_sb[ot][:, c * 512:(c + 1) * 512],
                                          ps[:])
            # v from xRT in natural layout [n, o], head-strided into vaug
            wv = [ld.tile([128, C], F32R, tag=f"w{i}", name=f"wv{i}")
                  for i in range(CT)]
            for i in range(CT):
                nc.gpsimd.dma_start(
                    wv[i][:], wqkv_d.ap()[i * 128:(i + 1) * 128, 2 * C:3 * C])
            for t in range(NT):
                for vc in range(2):
                    ps = qps.tile([128, 384], F32, tag="psv", name="vps", bufs=2)
                    for kt in range(CT):
                        nc.tensor.matmul(
                            ps[:], xRT[kt][:, t * 128:(t + 1) * 128],
                            wv[kt][:, vc * 384:(vc + 1) * 384],
                            start=(kt == 0), stop=(kt == CT - 1))
                    va_v = vaug[t][:].rearrange("p (h e) -> p h e", e=65)
                    ps_v = ps[:].rearrange("p (h d) -> p h d", d=64)
                    nc.vector.tensor_copy(va_v[:, vc * 6:(vc + 1) * 6, 0:64], ps_v)

        # ---------------- attention + proj ----------------
        with ExitStack() as esr:
            ldp = esr.enter_context(tc.tile_pool(name="ldp", bufs=1))
            pwT = [ldp.tile([128, C], F32R, tag=f"pw{i}", name=f"pw{i}")
                   for i in range(CT)]
            pbrow = ldp.tile([128, C], F32R, tag="pbrow")
            for i in range(CT):
                nc.gpsimd.dma_start(pwT[i][:],
                                    wproj_d.ap()[i * 128:(i + 1) * 128, :])
            nc.gpsimd.dma_start(pbrow[:], pbrep_d.ap()[:, :])

            with ExitStack() as esa:
                tblp = esa.enter_context(tc.tile_pool(name="tblp", bufs=3))
                ep = esa.enter_context(tc.tile_pool(name="expp", bufs=10))
                pp = esa.enter_context(tc.tile_pool(name="phat", bufs=10))
                tmpp = esa.enter_context(tc.tile_pool(name="tmp", bufs=4))
                sps = esa.enter_context(
                    tc.tile_pool(name="sps", bufs=2, space="PSUM"))
                avps = esa.enter_context(
                    tc.tile_pool(name="avps", bufs=4, space="PSUM"))

                for j in range(H // 2):
                    hpair = (2 * j, 2 * j + 1)
                    # TBLREP per head: TBL[p, J] = expG_h[63*(p//32)+p%32+J]
                    tbls = []
                    for h in hpair:
                        tblt = tblp.tile([128, TBLW], BF16, tag="tbl",
                                         name=f"tbl{h}")
                        for blk in range(4):
                            eng = nc.gpsimd if blk % 2 == 0 else nc.sync
                            eng.dma_start(
                                tblt[blk * 32:(blk + 1) * 32, :],
                                bass.AP(tbl_d, h * TW + 63 * blk,
                                        [[1, 32], [1, TBLW]]))
                        tbls.append(tblt)
                    avs = {h: [avps.tile([65, 512], F32, tag="av",
                                         name=f"av{h}_{c}") for c in range(2)]
                           for h in hpair}
                    for t in range(NT):
                        pss = [sps.tile([128, 1024], F32, tag="sc",
                                        name=f"sc{h}_{t}") for h in hpair]
                        # interleave the two heads' K=64 matmuls so they
                        # overlap in the PE array (row groups 0-1 vs 2-3)
                        for c in range(2):
                            for hi, h in enumerate(hpair):
                                qh = qk_sb[j][(h % 2) * 64:(h % 2) * 64 + 64, :]
                                kh = qk_sb[6 + j][(h % 2) * 64:(h % 2) * 64 + 64, :]
                                nc.tensor.matmul(
                                    pss[hi][:, c * 512:(c + 1) * 512],
                                    kh[:, t * 128:(t + 1) * 128],
                                    qh[:, c * 512:(c + 1) * 512],
                                    start=True, stop=True)
                        for hi, h in enumerate(hpair):
                            ee = ep.tile([128, 1024], BF16, tag="ee",
                                         name=f"ee{h}_{t}")
                            nc.scalar.activation(ee[:], pss[hi][:], EXP,
                                                 scale=SCALE)
                            tv = tbls[hi][:, 252 * t:252 * t + 2016].rearrange(
                                "p (c a b) -> p c a b", c=2, b=63)[:, :, :, :32]
                            ph = pp.tile([128, 1024], BF16, tag="ph",
                                         name=f"ph{h}_{t}")
                            pv = ph[:].rearrange("p (c a b) -> p c a b",
                                                 c=2, b=32)
                            ev = ee[:].rearrange("p (c a b) -> p c a b",
                                                 c=2, b=32)
                            nc.vector.tensor_mul(pv, ev, tv)
                            for c in range(2):
                                nc.tensor.matmul(
                                    avs[h][c][:],
                                    vaug[t][:, h * 65:(h + 1) * 65],
                                    ph[:, c * 512:(c + 1) * 512],
                                    start=(t == 0), stop=(t == NT - 1))
                    for h in hpair:
                        for c in range(2):
                            avsb = tmpp.tile([65, 512], F32, tag="avsb",
                                             name=f"avsb{h}{c}")
                            nc.vector.tensor_copy(avsb[:], avs[h][c][:])
                            rsb = tmpp.tile([128, 512], F32R, tag="rsb",
                                            name=f"rsb{h}{c}")
                            with nc.allow_low_precision(
                                    reason="softmax recip rounded to f32r"):
                                nc.vector.reciprocal(rsb[64:65, :],
                                                     avsb[64:65, :])
                            rep = avps.tile([64, 512], F32, tag="av",
                                            name=f"rep{h}{c}")
                            nc.tensor.matmul(rep[:], ones_r[64:65, 0:64],
                                             rsb[64:65, :],
                                             start=True, stop=True)
                            dst = outT[h // 2][(h % 2) * 64:(h % 2) * 64 + 64,
                                               c * 512:(c + 1) * 512]
                            if h % 2 == 0:
                                nc.vector.tensor_mul(dst, avsb[0:64, :],
                                                     rep[:])
                            else:
                                tmp = tmpp.tile([64, 512], F32R, tag="tmo",
                                                name=f"tmo{h}{c}")
                                nc.vector.tensor_mul(tmp[:], avsb[0:64, :],
                                                     rep[:])
                                nc.sync.dma_start(dst, tmp[:])

            # ---------------- proj ----------------
            with ExitStack() as esp:
                pjps = esp.enter_context(
                    tc.tile_pool(name="pjps", bufs=4, space="PSUM"))
                fsb = esp.enter_context(tc.tile_pool(name="fsb", bufs=4))
                for t in range(NT):
                    f = fsb.tile([128, C], F32, tag="f", name=f"f{t}")
                    for pc in range(2):
                        ps = pjps.tile([128, 384], F32, tag="ps", name="pjps")
                        for kt in range(CT):
                            nc.tensor.matmul(
                                ps[:], outT[kt][:, t * 128:(t + 1) * 128],
                                pwT[kt][:, pc * 384:(pc + 1) * 384],
                                start=(kt == 0), stop=False)
                        # + proj bias via K=1 matmul accumulate
                        nc.tensor.matmul(
                            ps[:], ones_r[0:1, 0:128],
                            pbrow[0:1, pc * 384:(pc + 1) * 384],
                            start=False, stop=True)
                        nc.vector.tensor_copy(f[:, pc * 384:(pc + 1) * 384],
                                              ps[:])
                    nc.sync.dma_start(out_d.ap()[t * 128:(t + 1) * 128, :], f[:])


_GRAPH = None


def _graph():
    global _GRAPH
    if _GRAPH is None:
        _GRAPH = _build_graph()
    return _GRAPH


def _host_prep(x, qkv_w, proj_w, proj_b, rpb_w1, rpb_b1, rpb_w2, rpb_b2):
    """Numpy layout prep + exp of the 63x63 bias table (7 MFLOP)."""
    a = np.arange(63, dtype=np.float32) - 31.0
    rel_y = np.broadcast_to(a[:, None], (63, 63))
    rel_x = np.broadcast_to(a[None, :], (63, 63))
    rel = np.stack([rel_x, rel_y], -1).reshape(-1, 2)           # [3969, 2]
    hdn = np.maximum(rel @ rpb_w1.T + rpb_b1, 0.0)
    gtbl = (hdn @ rpb_w2.T + rpb_b2).T.astype(np.float32)       # [12, 3969]
    gtbl = np.exp(gtbl, dtype=np.float32)                       # exp(bias)
    import ml_dtypes
    gpad = np.zeros((H, TW), np.float32)
    gpad[:, :3969] = gtbl
    gpad = gpad.astype(ml_dtypes.bfloat16)

    wqkvT = np.ascontiguousarray(qkv_w.T.astype(np.float32))    # [768, 2304]
    wprojT = np.ascontiguousarray(proj_w.T.astype(np.float32))  # [768, 768]
    pbrep = np.ascontiguousarray(
        np.broadcast_to(proj_b.astype(np.float32), (128, C)))
    shared = {"qkv_wT": wqkvT, "proj_wT": wprojT, "proj_b_rep": pbrep,
              "rpb_tbl": gpad}
    in_maps = []
    for i in range(B):
        m = dict(shared)
        m["xT"] = np.ascontiguousarray(x[i].T.astype(np.float32))
        m["xRT"] = np.ascontiguousarray(x[i][::-1].T.astype(np.float32))
        in_maps.append(m)
    return in_maps


def kernel(x, qkv_w, proj_w, proj_b, rpb_w1, rpb_b1, rpb_w2, rpb_b2,
           _trace=False, _tmpdir=None):
    in_maps = _host_prep(np.asarray(x), np.asarray(qkv_w), np.asarray(proj_w),
                         np.asarray(proj_b), np.asarray(rpb_w1),
                         np.asarray(rpb_b1), np.asarray(rpb_w2),
                         np.asarray(rpb_b2))
    nc = _graph()
    res = run_bass_kernel_spmd(nc, in_maps, core_ids=list(range(B)),
                               trace=_trace, tmpdir=_tmpdir)
    out = np.stack([res.results[i]["out"] for i in range(B)])
    if _trace:
        kernel._last_results = res
    return out



# revision 38
# speedup vs baseline: 1.2427x; 1.2427x over previous
"""Multi-head attention with relative-position-bias MLP on 8 TRN2 NeuronCores.

Strategy: pure data-parallel over batch (B=8 -> 1 batch element per core, no
collectives). All matmul operands are bf16 (fp8 was measured to fail the 2e-2
gate: softmax does not attenuate logit noise relative to output magnitude).
Host-side prep is layout only: per-core transposed x (plus a token-reversed
copy feeding k/v), transposed weights, replicated proj bias, and exp() of the
63x63 relative-position bias table.

Device algorithm per core (N=1024 tokens, C=768, H=12 heads, D=64):
  qT[o,n] = wqkv[:, o].T @ xT        (per head-pair o-block, bf16)
  kT[o,n] = wqkv[:, C+o].T @ xRT     (token-reversed k)
  v[n,o]  = xRT.T @ wqkv[:, 2C+o]    (token-reversed v, + ones column)
  per head h, k-tile t (128 reversed tokens):
     sT_h = kT_h(t).T @ qT_h          [nk=128, nq=1024]
     E = exp(sT * SCALE)              (ACT, PSUM->SBUF bf16)
     P = E * expB_tile                (DVE, all-SBUF bf16;
                                       exp(s+b) = exp(s)*exp(b))
     av[q, h-slot] += P(:, qtile).T @ [v_h(t) | 1]   (flipped AV: q on
                                       partitions, 65 free rows per matmul --
                                       half the PE rows of the [65, q] layout)
  normalize: recip of the ones-column sums, per-partition tensor_scalar mul
  (PSUM->SBUF bf16) -- no PE broadcast needed since the denominator is a
  per-q-partition scalar in this layout.
  transpose [q, c] -> [c, q] via the DMA crossbar (dma_start_transpose,
  16x128 xbar tiles; no PE or DVE cost) to feed proj's stationary side.
  final = outT.T @ projT (+ proj_b via a DVE add on the PSUM->SBUF copy)

Token reversal trick: bias[h,n,m] depends on grid coords of (n,m) only via
(cy_n - cy_m, cx_n - cx_m). Reversing key/value token order makes the
Toeplitz expansion all-positive-stride: TBLREP_h[p, J] = expG_h[63*(p//32)
+ p%32 + J] (4 plain DMAs per head), and each [128,1024] bias tile is a
strided view of it. The AV reduction over k-tiles is order-invariant.

Schedule: five single-bank PSUM ring tiles (a tile pool tag-ring, so Tile
pipelines producers against consumers) carry score chunks and the qk / v /
proj production groups; a 3-bank accumulator holds one head-pair's AV, with
all its 6-slots-per-bank cleared by one full-bank zero matmul per bank
(start=True zeroes the whole 2KB ZERO_REGION, so per-slot starts would erase
siblings). Cascaded AV deferral: pair j's 16 P tiles stay resident in SBUF
and its AV batches run during pair j+1, so the steady-state per-pair PE work
(scores + previous pair's AV) sits well under the ACT exp pace that bounds
each pair; normalize/transpose for pair j happen at pair j+1's end, and a
phantom stage after pair 5 finishes the last pair interleaved with proj.
v-production runs inside pairs 0-1; pair j+1's q/k production and bias-table
DMAs are interleaved into pair j; a dedicated first-pair weight slice plus
warm-up matmuls (PE p-state: 2x cycle cost for ~3us after idle) shorten the
DMA-bound startup. Loads: x via Pool SWDGE, weights/tables via SP HWDGE
(parallel issue paths; consolidated one-DMA-per-tensor layouts).
"""
import sys

import numpy as np

sys.path.insert(0, "/opt/trn_rl_repo")

import concourse.bass as bass  # noqa: E402
import concourse.mybir as mybir  # noqa: E402
import concourse.tile as tile  # noqa: E402
from concourse import bacc  # noqa: E402
from concourse.bass_utils import run_bass_kernel_spmd  # noqa: E402

F32 = mybir.dt.float32
BF16 = mybir.dt.bfloat16
EXP = mybir.ActivationFunctionType.Exp

B, N, C, H, D = 8, 1024, 768, 12, 64
SCALE = float(D) ** -0.5
NT = N // 128   # 8 token tiles
CT = C // 128   # 6 channel tiles
TBLW = 3781     # TBLREP width (padded so 2016-wide views stay in range)
TW = 4001       # DRAM table width per head (>= 220 + TBLW, zero-padded)


def _slot(hi, qt):
    """Free-offset (f32 elems) of head hi / q-tile qt in the AV accumulator.

    3 q-tiles per 2KB PSUM bank (170-elem stride), heads 85 apart, so every
    65-wide matmul output stays inside one bank.
    """
    return (qt // 3) * 512 + (qt % 3) * 170 + 85 * hi


def _rcol(hi, qt):
    return (qt // 3) * 6 + (qt % 3) * 2 + hi


def _build_graph():
    nc = bacc.Bacc("TRN2", target_bir_lowering=False, debug=False,
                   enable_asserts=False, num_devices=B)
    xT_d = nc.dram_tensor("xT", [128, CT * N], BF16, kind="ExternalInput")
    xRT_d = nc.dram_tensor("xRT", [128, CT * N], BF16,
                           kind="ExternalInput")
    wqk_d = nc.dram_tensor("qk_wT", [128, CT * 2 * C], BF16,
                           kind="ExternalInput")
    wqk0_d = nc.dram_tensor("qk0_wT", [128, CT * 256], BF16,
                            kind="ExternalInput")
    wv_d = nc.dram_tensor("v_wT", [128, CT * C], BF16, kind="ExternalInput")
    wproj_d = nc.dram_tensor("proj_wT", [128, CT * C], BF16,
                             kind="ExternalInput")
    pbrep_d = nc.dram_tensor("proj_b_rep", [128, C], F32, kind="ExternalInput")
    tbl_d = nc.dram_tensor("rpb_tbl", [H, TW], BF16, kind="ExternalInput")
    out_d = nc.dram_tensor("out", [N, C], F32, kind="ExternalOutput")

    with tile.TileContext(nc) as tc:
        _kern(tc, nc, xT_d, xRT_d, wqk_d, wqk0_d, wv_d, wproj_d, pbrep_d,
              tbl_d, out_d)
    nc.compile()
    return nc


def _kern(tc, nc, xT_d, xRT_d, wqk_d, wqk0_d, wv_d, wproj_d, pbrep_d,
          tbl_d, out_d):
    from contextlib import ExitStack

    with ExitStack() as es:
        persist = es.enter_context(tc.tile_pool(name="persist", bufs=1))
        xTt = persist.tile([128, CT * N], BF16, tag="xt", name="xt")
        xRTt = persist.tile([128, CT * N], BF16, tag="xr", name="xr")
        wqkt = persist.tile([128, CT * 2 * C], BF16, tag="w", name="w")
        wvt = persist.tile([128, CT * C], BF16, tag="wv", name="wv")
        pwt = persist.tile([128, CT * C], BF16, tag="pw", name="pw")
        wqk0t = persist.tile([128, CT * 256], BF16, tag="w0", name="w0")
        xT = [xTt[:, i * N:(i + 1) * N] for i in range(CT)]
        xRT = [xRTt[:, i * N:(i + 1) * N] for i in range(CT)]
        wqk = [wqkt[:, i * 2 * C:(i + 1) * 2 * C] for i in range(CT)]
        wv = [wvt[:, i * C:(i + 1) * C] for i in range(CT)]
        pw = [pwt[:, i * C:(i + 1) * C] for i in range(CT)]
        pbrow = persist.tile([128, C], F32, tag="pb", name="pbrow")
        qs = [persist.tile([128, N], BF16, tag=f"q{j}", name=f"q{j}")
              for j in range(H // 2)]
        ks = [persist.tile([128, N], BF16, tag=f"k{j}", name=f"k{j}")
              for j in range(H // 2)]
        vaug = [persist.tile([128, H * 65], BF16, tag=f"va{t}", name=f"va{t}")
                for t in range(NT)]
        outT = [persist.tile([128, N], BF16, tag=f"ot{i}", name=f"ot{i}")
                for i in range(CT)]

        tblp = es.enter_context(tc.tile_pool(name="tblp", bufs=4))
        eep = es.enter_context(tc.tile_pool(name="eep", bufs=4))
        php = es.enter_context(tc.tile_pool(name="php", bufs=18))
        onp = es.enter_context(tc.tile_pool(name="onp", bufs=2))
        rsp = es.enter_context(tc.tile_pool(name="rsp", bufs=2))
        fp = es.enter_context(tc.tile_pool(name="fp", bufs=3))
        # 5 single-bank PSUM tiles shared (tag-ring) by score chunks and the
        # qk / v / proj production groups: distinct tiles per allocation is
        # what lets Tile pipeline producers against consumers
        ringp = es.enter_context(
            tc.tile_pool(name="ring", bufs=5, space="PSUM"))
        avp = es.enter_context(tc.tile_pool(name="avp", bufs=1, space="PSUM"))

        def ring_tile(name):
            return ringp.tile([128, 512], F32, tag="rg", name=name, bufs=5)

        # ones column of the augmented-v tiles (denominator rides the AV
        # matmul's 65th free column)
        for t in range(NT):
            va = vaug[t][:].rearrange("p (h u) -> p h u", u=65)
            nc.gpsimd.memset(va[:, :, 64:65], 1.0)
        # zero operands for the AV-bank clearing matmuls: a start=True matmul
        # zeroes its whole 2KB PSUM bank (ZERO_REGION), so the 6 slots sharing
        # a bank must be cleared by one full-bank matmul, not per-slot starts
        z1 = persist.tile([1, 128], BF16, tag="z1", name="z1")
        nc.gpsimd.memset(z1[:], 0.0)
        z512 = persist.tile([1, 512], BF16, tag="z512", name="z512")
        nc.gpsimd.memset(z512[:], 0.0)

        # ---- loads: one consolidated DMA per tensor class; x on the Pool
        # SWDGE path, weights/tables on the SP HWDGE path (parallel issue).
        # While they are in flight, dummy matmuls keep the PE busy so its
        # p-state ramp (2x cycle cost for the first 3us after idle) is paid
        # on junk work instead of the first real groups. ----
        def warm(n, label):
            for wi in range(n):
                wt = ring_tile(f"{label}{wi}")
                nc.tensor.matmul(wt[:], z1[:], z512[:], start=True, stop=True)

        warm(30, "warm")
        nc.sync.dma_start(wqk0t[:], wqk0_d.ap()[:, :])
        nc.gpsimd.dma_start(xTt[:], xT_d.ap()[:, :])
        nc.gpsimd.dma_start(xRTt[:], xRT_d.ap()[:, :])

        def load_tbl(h):
            t = tblp.tile([128, TBLW], BF16, tag="tbl", name=f"tbl{h}")
            for blk in range(4):
                nc.sync.dma_start(
                    t[blk * 32:(blk + 1) * 32, :],
                    bass.AP(tbl_d, h * TW + 63 * blk, [[1, 32], [1, TBLW]]))
            return t

        tbls = {0: load_tbl(0)}
        nc.sync.dma_start(wvt[:], wv_d.ap()[:, :])
        tbls[1] = load_tbl(1)
        nc.sync.dma_start(wqkt[:], wqk_d.ap()[:, :])

        # proj weights late: they are only needed at the tail
        nc.sync.dma_start(pwt[:], wproj_d.ap()[:, :])
        nc.sync.dma_start(pbrow[:], pbrep_d.ap()[:, :])

        def qk_group(j, sec):
            """Produce q (sec=0) or k (sec=1) for head-pair j: [128 o, N]."""
            rhs = xT if sec == 0 else xRT
            dst = (qs if sec == 0 else ks)[j][:]
            for c in range(2):
                ps = ring_tile(f"qk{sec}_{j}_{c}")
                for kt in range(CT):
                    if j == 0:
                        wsl = wqk0t[:, kt * 256 + sec * 128:
                                    kt * 256 + sec * 128 + 128]
                    else:
                        wsl = wqk[kt][:, sec * C + j * 128:
                                      sec * C + j * 128 + 128]
                    nc.tensor.matmul(ps[:], wsl,
                                     rhs[kt][:, c * 512:(c + 1) * 512],
                                     start=(kt == 0), stop=(kt == CT - 1))
                nc.vector.tensor_copy(dst[:, c * 512:(c + 1) * 512], ps[:])

        def v_group(t):
            """v rows for (reversed) token tile t: [128 n, 768 o] -> vaug."""
            dst = vaug[t][:].rearrange("p (h u) -> p h u", u=65)[:, :, 0:64]
            dst = dst.rearrange("p (a g) d -> p a g d", a=2)
            for vc in range(2):
                ps = ring_tile(f"v{t}_{vc}")
                for kt in range(CT):
                    nc.tensor.matmul(ps[:, 0:384],
                                     xRT[kt][:, t * 128:(t + 1) * 128],
                                     wv[kt][:, vc * 384:(vc + 1) * 384],
                                     start=(kt == 0), stop=(kt == CT - 1))
                nc.vector.tensor_copy(
                    dst[:, vc],
                    ps[:, 0:384].rearrange("p (g d) -> p g d", d=64))

        qk_group(0, 0)
        qk_group(0, 1)

        # ---------------- attention ----------------
        # Cascaded AV deferral: pair j's AV matmuls run during pair j+1 (all
        # 16 P tiles of a pair stay resident in SBUF), so the per-pair PE work
        # in steady state is just scores + the previous pair's AV -- well
        # under the ACT (exp) pace that bounds each pair. The single 3-bank
        # accumulator still suffices: pair j's accumulation window is pair
        # j+1, released by the normalize at pair j+1's end.
        def finish_pair(pj, avf, phs):
            """Normalize + transpose for pair pj (AV already accumulated)."""
            rsb = rsp.tile([128, 18], F32, tag="rs", name=f"rs{pj}")
            den = avf.rearrange("p (g x) -> p g x", g=3)[:, :, 0:510]
            den = den.rearrange("p g (r y) -> p g r y", y=170)
            den = den.rearrange("p g r (h z) -> p g r h z", z=85)
            den = den[:, :, :, :, 64:65]
            rv = rsb[:].rearrange("p (g r h) -> p g r h", g=3, r=3)
            with nc.allow_low_precision(reason="softmax recip in f32"):
                nc.vector.reciprocal(rv.unsqueeze(-1), den)
            on = onp.tile([128, N], BF16, tag="on", name=f"on{pj}")
            for g in range(3):
                rc = 3 if g < 2 else 2
                src_v = avf[:, g * 512:g * 512 + 170 * rc].rearrange(
                    "p (r z) -> p r z", z=170)
                src_v = src_v.rearrange("p r (h y) -> p r h y",
                                        y=85)[:, :, :, 0:64]
                dst_v = on[:, g * 384:g * 384 + 128 * rc].rearrange(
                    "p (r h d) -> p r h d", h=2, d=64)
                sc_v = rsb[:, g * 6:g * 6 + 2 * rc].rearrange(
                    "p (r h) -> p r h", h=2).unsqueeze(-1)
                sc_v = sc_v.broadcast_to([128, rc, 2, 64])
                nc.vector.tensor_mul(dst_v, src_v, sc_v)
            # [q, c'] -> [c', q] through the DMA crossbar
            nc.sync.dma_start_transpose(
                outT[pj][:].rearrange("p (a b) -> p a b", b=128), on[:])

        def new_av(pj):
            av = avp.tile([128, 1536], F32, tag="av", name=f"av{pj}")
            avf = av[:]
            for g in range(3):
                nc.tensor.matmul(avf[:, g * 512:(g + 1) * 512], z1[:],
                                 z512[:], start=True, stop=True)
            return avf

        def av_batch(avf, pj, phs, hi, t):
            h = 2 * pj + hi
            ph = phs[hi][t]
            for qt in range(NT):
                so = _slot(hi, qt)
                nc.tensor.matmul(avf[:, so:so + 65],
                                 ph[:, qt * 128:(qt + 1) * 128],
                                 vaug[t][:, h * 65:h * 65 + 65],
                                 start=False, stop=(t == NT - 1),
                                 skip_group_check=True)

        prev_phs = None
        avf = None
        for j in range(H // 2):
            if j > 0:
                avf = new_av(j - 1)
            cur_phs = {0: [], 1: []}
            for t in range(NT):
                if t == 0 and j < 5:
                    tbls[2 * j + 2] = load_tbl(2 * j + 2)
                    tbls[2 * j + 3] = load_tbl(2 * j + 3)
                for hi in range(2):
                    h = 2 * j + hi
                    ee = eep.tile([128, N], BF16, tag="ee", name=f"ee{h}_{t}")
                    for c in range(2):
                        ps = ring_tile(f"sc{h}_{t}_{c}")
                        nc.tensor.matmul(
                            ps[:],
                            ks[j][hi * 64:hi * 64 + 64, t * 128:(t + 1) * 128],
                            qs[j][hi * 64:hi * 64 + 64, c * 512:(c + 1) * 512],
                            start=True, stop=True)
                        nc.scalar.activation(ee[:, c * 512:(c + 1) * 512],
                                             ps[:], EXP, scale=SCALE)
                    if j > 0:
                        av_batch(avf, j - 1, prev_phs, hi, t)
                    ph = php.tile([128, N], BF16, tag="ph", name=f"ph{h}_{t}")
                    tv = tbls[h][:, 252 * t:252 * t + 2016].rearrange(
                        "p (c a b) -> p c a b", c=2, b=63)[:, :, :, 0:32]
                    ev = ee[:].rearrange("p (c a b) -> p c a b", c=2, b=32)
                    pv = ph[:].rearrange("p (c a b) -> p c a b", c=2, b=32)
                    nc.vector.tensor_mul(pv, ev, tv)
                    cur_phs[hi].append(ph)
                # interleaved production for upcoming consumers
                if j == 0 and t < 6:
                    v_group(t)
                if j == 1 and t in (5, 6):
                    v_group(t + 1)
                if j < 5 and t == 5:
                    qk_group(j + 1, 0)
                if j < 5 and t == 6:
                    qk_group(j + 1, 1)
            if j > 0:
                finish_pair(j - 1, avf, prev_phs)
            prev_phs = cur_phs

        # phantom pair: accumulate + finish pair 5 (proj groups for the
        # same t are interleaved so the PE chews their kt<4 steps while the
        # last transposes land)
        avf = new_av(5)

        for t in range(NT):
            for hi in range(2):
                av_batch(avf, 5, prev_phs, hi, t)
        finish_pair(5, avf, prev_phs)

        # ---------------- proj ----------------
        warm(14, "fill")
        for t in range(NT):
            f = fp.tile([128, C], F32, tag="f", name=f"f{t}")
            for pc in range(2):
                ps = ring_tile(f"pj{t}_{pc}")
                for kt in range(CT):
                    nc.tensor.matmul(ps[:, 0:384],
                                     outT[kt][:, t * 128:(t + 1) * 128],
                                     pw[kt][:, pc * 384:(pc + 1) * 384],
                                     start=(kt == 0), stop=(kt == CT - 1))
                nc.vector.tensor_add(f[:, pc * 384:(pc + 1) * 384],
                                     ps[:, 0:384],
                                     pbrow[:, pc * 384:(pc + 1) * 384])
            nc.sync.dma_start(out_d.ap()[t * 128:(t + 1) * 128, :], f[:])


_GRAPH = None


def _graph():
    global _GRAPH
    if _GRAPH is None:
        _GRAPH = _build_graph()
    return _GRAPH


def _host_prep(x, qkv_w, proj_w, proj_b, rpb_w1, rpb_b1, rpb_w2, rpb_b2):
    """Numpy layout prep + exp of the 63x63 bias table (7 MFLOP)."""
    import ml_dtypes
    a = np.arange(63, dtype=np.float32) - 31.0
    rel_y = np.broadcast_to(a[:, None], (63, 63))
    rel_x = np.broadcast_to(a[None, :], (63, 63))
    rel = np.stack([rel_x, rel_y], -1).reshape(-1, 2)           # [3969, 2]
    hdn = np.maximum(rel @ rpb_w1.T + rpb_b1, 0.0)
    gtbl = (hdn @ rpb_w2.T + rpb_b2).T.astype(np.float32)       # [12, 3969]
    gtbl = np.exp(gtbl, dtype=np.float32)                       # exp(bias)
    gpad = np.zeros((H, TW), np.float32)
    gpad[:, :3969] = gtbl
    gpad = gpad.astype(ml_dtypes.bfloat16)

    bf = ml_dtypes.bfloat16

    def fold(a):
        """[C, W] -> [128, CT*W]: channel tile kt becomes a free-dim block."""
        w = a.shape[1]
        return np.ascontiguousarray(
            a.reshape(CT, 128, w).transpose(1, 0, 2).reshape(128, CT * w))

    wqkvT = qkv_w.T.astype(bf)                                  # [768, 2304]
    wqkT = fold(wqkvT[:, 0:2 * C])
    wqk0 = np.ascontiguousarray(np.concatenate(
        [wqkvT[:, 0:128], wqkvT[:, C:C + 128]],
        axis=1).reshape(CT, 128, 256).transpose(1, 0, 2).reshape(128, -1))
    wvT = fold(wqkvT[:, 2 * C:3 * C])
    wprojT = fold(proj_w.T.astype(bf))                          # [768, 768]
    pbrep = np.ascontiguousarray(
        np.broadcast_to(proj_b.astype(np.float32), (128, C)))
    shared = {"qk_wT": wqkT, "qk0_wT": wqk0, "v_wT": wvT, "proj_wT": wprojT,
              "proj_b_rep": pbrep, "rpb_tbl": gpad}
    in_maps = []
    for i in range(B):
        m = dict(shared)
        m["xT"] = fold(x[i].T.astype(bf))
        m["xRT"] = fold(x[i][::-1].T.astype(bf))
        in_maps.append(m)
    return in_maps


def kernel(x, qkv_w, proj_w, proj_b, rpb_w1, rpb_b1, rpb_w2, rpb_b2,
           _trace=False, _tmpdir=None):
    in_maps = _host_prep(np.asarray(x), np.asarray(qkv_w), np.asarray(proj_w),
                         np.asarray(proj_b), np.asarray(rpb_w1),
                         np.asarray(rpb_b1), np.asarray(rpb_w2),
                         np.asarray(rpb_b2))
    nc = _graph()
    res = run_bass_kernel_spmd(nc, in_maps, core_ids=list(range(B)),
                               trace=_trace, tmpdir=_tmpdir)
    out = np.stack([res.results[i]["out"] for i in range(B)])
    if _trace:
        kernel._last_results = res
    return out


# revision 44
# speedup vs baseline: 1.2926x; 1.0402x over previous
"""Multi-head attention with relative-position-bias MLP on 8 TRN2 NeuronCores.

Strategy: pure data-parallel over batch (B=8 -> 1 batch element per core, no
collectives). All matmul operands are bf16 (fp8 was measured to fail the 2e-2
gate: softmax does not attenuate logit noise relative to output magnitude).
Host-side prep is layout only: per-core transposed x (plus a token-reversed
copy feeding k/v), transposed weights, replicated proj bias, and exp() of the
63x63 relative-position bias table.

Device algorithm per core (N=1024 tokens, C=768, H=12 heads, D=64):
  qT[o,n] = wqkv[:, o].T @ xT        (per head-pair o-block, bf16)
  kT[o,n] = wqkv[:, C+o].T @ xRT     (token-reversed k)
  v[n,o]  = xRT.T @ wqkv[:, 2C+o]    (token-reversed v, + ones column)
  per head h, k-tile t (128 reversed tokens):
     sT_h = kT_h(t).T @ qT_h          [nk=128, nq=1024]
     E = exp(sT * SCALE)              (ACT, PSUM->SBUF bf16)
     P = E * expB_tile                (DVE, all-SBUF bf16;
                                       exp(s+b) = exp(s)*exp(b))
     av[q, h-slot] += P(:, qtile).T @ [v_h(t) | 1]   (flipped AV: q on
                                       partitions, 65 free rows per matmul --
                                       half the PE rows of the [65, q] layout)
  normalize: recip of the ones-column sums, per-partition tensor_scalar mul
  (PSUM->SBUF bf16) -- no PE broadcast needed since the denominator is a
  per-q-partition scalar in this layout.
  transpose [q, c] -> [c, q] via the DMA crossbar (dma_start_transpose,
  16x128 xbar tiles; no PE or DVE cost) to feed proj's stationary side.
  final = outT.T @ projT (+ proj_b via a DVE add on the PSUM->SBUF copy)

Token reversal trick: bias[h,n,m] depends on grid coords of (n,m) only via
(cy_n - cy_m, cx_n - cx_m). Reversing key/value token order makes the
Toeplitz expansion all-positive-stride: TBLREP_h[p, J] = expG_h[63*(p//32)
+ p%32 + J] (4 plain DMAs per head), and each [128,1024] bias tile is a
strided view of it. The AV reduction over k-tiles is order-invariant.

Schedule: five single-bank PSUM ring tiles (a tile pool tag-ring, so Tile
pipelines producers against consumers) carry score chunks and the qk / v /
proj production groups; a 3-bank accumulator holds one head-pair's AV, with
all its 6-slots-per-bank cleared by one full-bank zero matmul per bank
(start=True zeroes the whole 2KB ZERO_REGION, so per-slot starts would erase
siblings). Cascaded AV deferral: pair j's 16 P tiles stay resident in SBUF
and its AV batches run during pair j+1, so the steady-state per-pair PE work
(scores + previous pair's AV) sits well under the ACT exp pace that bounds
each pair; normalize/transpose for pair j happen at pair j+1's end, and a
phantom stage after pair 5 finishes the last pair interleaved with proj.
v-production runs inside pairs 0-1; pair j+1's q/k production and bias-table
DMAs are interleaved into pair j; a dedicated first-pair weight slice plus
warm-up matmuls (PE p-state: 2x cycle cost for ~3us after idle) shorten the
DMA-bound startup. Loads: x via Pool SWDGE, weights/tables via SP HWDGE
(parallel issue paths; consolidated one-DMA-per-tensor layouts).
"""
import sys

import numpy as np

sys.path.insert(0, "/opt/trn_rl_repo")

import concourse.bass as bass  # noqa: E402
import concourse.mybir as mybir  # noqa: E402
import concourse.tile as tile  # noqa: E402
from concourse import bacc  # noqa: E402
from concourse.bass_utils import run_bass_kernel_spmd  # noqa: E402

F32 = mybir.dt.float32
BF16 = mybir.dt.bfloat16
EXP = mybir.ActivationFunctionType.Exp

B, N, C, H, D = 8, 1024, 768, 12, 64
SCALE = float(D) ** -0.5
NT = N // 128   # 8 token tiles
CT = C // 128   # 6 channel tiles
TBLW = 3781     # TBLREP width (padded so 2016-wide views stay in range)
TW = 4001       # DRAM table width per head (>= 220 + TBLW, zero-padded)


def _slot(hi, qt):
    """Free-offset (f32 elems) of head hi / q-tile qt in the AV accumulator.

    3 q-tiles per 2KB PSUM bank (170-elem stride), heads 85 apart, so every
    65-wide matmul output stays inside one bank.
    """
    return (qt // 3) * 512 + (qt % 3) * 170 + 85 * hi


def _rcol(hi, qt):
    return (qt // 3) * 6 + (qt % 3) * 2 + hi


def _build_graph():
    nc = bacc.Bacc("TRN2", target_bir_lowering=False, debug=False,
                   enable_asserts=False, num_devices=B)
    xT_d = nc.dram_tensor("xT", [128, CT * N], BF16, kind="ExternalInput")
    xRT_d = nc.dram_tensor("xRT", [128, CT * N], BF16,
                           kind="ExternalInput")
    wqk_d = nc.dram_tensor("qk_wT", [128, CT * 2 * C], BF16,
                           kind="ExternalInput")
    wqk0_d = nc.dram_tensor("qk0_wT", [128, CT * 256], BF16,
                            kind="ExternalInput")
    wv_d = nc.dram_tensor("v_wT", [128, CT * C], BF16, kind="ExternalInput")
    wproj_d = nc.dram_tensor("proj_wT", [128, CT * C], BF16,
                             kind="ExternalInput")
    pbrep_d = nc.dram_tensor("proj_b_rep", [128, C], F32, kind="ExternalInput")
    tbl_d = nc.dram_tensor("rpb_tbl", [H, TW], BF16, kind="ExternalInput")
    out_d = nc.dram_tensor("out", [N, C], F32, kind="ExternalOutput")

    with tile.TileContext(nc) as tc:
        _kern(tc, nc, xT_d, xRT_d, wqk_d, wqk0_d, wv_d, wproj_d, pbrep_d,
              tbl_d, out_d)
    nc.compile()
    return nc


def _kern(tc, nc, xT_d, xRT_d, wqk_d, wqk0_d, wv_d, wproj_d, pbrep_d,
          tbl_d, out_d):
    from contextlib import ExitStack

    with ExitStack() as es:
        persist = es.enter_context(tc.tile_pool(name="persist", bufs=1))
        xTt = persist.tile([128, CT * N], BF16, tag="xt", name="xt")
        xRTt = persist.tile([128, CT * N], BF16, tag="xr", name="xr")
        wqkt = persist.tile([128, CT * 2 * C], BF16, tag="w", name="w")
        wvt = persist.tile([128, CT * C], BF16, tag="wv", name="wv")
        pwt = persist.tile([128, CT * C], BF16, tag="pw", name="pw")
        wqk0t = persist.tile([128, CT * 256], BF16, tag="w0", name="w0")
        xT = [xTt[:, i * N:(i + 1) * N] for i in range(CT)]
        xRT = [xRTt[:, i * N:(i + 1) * N] for i in range(CT)]
        wqk = [wqkt[:, i * 2 * C:(i + 1) * 2 * C] for i in range(CT)]
        wv = [wvt[:, i * C:(i + 1) * C] for i in range(CT)]
        pw = [pwt[:, i * C:(i + 1) * C] for i in range(CT)]
        pbrow = persist.tile([128, C], F32, tag="pb", name="pbrow")
        qs = [persist.tile([128, N], BF16, tag=f"q{j}", name=f"q{j}")
              for j in range(H // 2)]
        ks = [persist.tile([128, N], BF16, tag=f"k{j}", name=f"k{j}")
              for j in range(H // 2)]
        vaug = [persist.tile([128, H * 65], BF16, tag=f"va{t}", name=f"va{t}")
                for t in range(NT)]
        outT = [persist.tile([128, N], BF16, tag=f"ot{i}", name=f"ot{i}")
                for i in range(CT)]

        tblp = es.enter_context(tc.tile_pool(name="tblp", bufs=4))
        eep = es.enter_context(tc.tile_pool(name="eep", bufs=4))
        php = es.enter_context(tc.tile_pool(name="php", bufs=19))
        onp = es.enter_context(tc.tile_pool(name="onp", bufs=2))
        rsp = es.enter_context(tc.tile_pool(name="rsp", bufs=2))
        fp = es.enter_context(tc.tile_pool(name="fp", bufs=3))
        # 5 single-bank PSUM tiles shared (tag-ring) by score chunks and the
        # qk / v / proj production groups: distinct tiles per allocation is
        # what lets Tile pipeline producers against consumers
        ringp = es.enter_context(
            tc.tile_pool(name="ring", bufs=5, space="PSUM"))
        avp = es.enter_context(tc.tile_pool(name="avp", bufs=1, space="PSUM"))

        def ring_tile(name):
            return ringp.tile([128, 512], F32, tag="rg", name=name, bufs=5)

        # ones column of the augmented-v tiles (denominator rides the AV
        # matmul's 65th free column)
        for t in range(NT):
            va = vaug[t][:].rearrange("p (h u) -> p h u", u=65)
            nc.gpsimd.memset(va[:, :, 64:65], 1.0)
        # zero operands for the AV-bank clearing matmuls: a start=True matmul
        # zeroes its whole 2KB PSUM bank (ZERO_REGION), so the 6 slots sharing
        # a bank must be cleared by one full-bank matmul, not per-slot starts
        z1 = persist.tile([1, 128], BF16, tag="z1", name="z1")
        nc.gpsimd.memset(z1[:], 0.0)
        z512 = persist.tile([1, 512], BF16, tag="z512", name="z512")
        nc.gpsimd.memset(z512[:], 0.0)

        # ---- loads: one consolidated DMA per tensor class; x on the Pool
        # SWDGE path, weights/tables on the SP HWDGE path (parallel issue).
        # While they are in flight, dummy matmuls keep the PE busy so its
        # p-state ramp (2x cycle cost for the first 3us after idle) is paid
        # on junk work instead of the first real groups. ----
        def warm(n, label):
            for wi in range(n):
                wt = ring_tile(f"{label}{wi}")
                nc.tensor.matmul(wt[:], z1[:], z512[:], start=True, stop=True)

        warm(30, "warm")
        nc.sync.dma_start(wqk0t[:], wqk0_d.ap()[:, :])
        nc.gpsimd.dma_start(xTt[:], xT_d.ap()[:, :])
        nc.gpsimd.dma_start(xRTt[:], xRT_d.ap()[:, :])

        def load_tbl(h):
            t = tblp.tile([128, TBLW], BF16, tag="tbl", name=f"tbl{h}")
            for blk in range(4):
                nc.sync.dma_start(
                    t[blk * 32:(blk + 1) * 32, :],
                    bass.AP(tbl_d, h * TW + 63 * blk, [[1, 32], [1, TBLW]]))
            return t

        tbls = {0: load_tbl(0)}
        nc.sync.dma_start(wvt[:], wv_d.ap()[:, :])
        tbls[1] = load_tbl(1)
        nc.sync.dma_start(wqkt[:], wqk_d.ap()[:, :])

        # proj weights late: they are only needed at the tail
        nc.sync.dma_start(pwt[:], wproj_d.ap()[:, :])
        nc.sync.dma_start(pbrow[:], pbrep_d.ap()[:, :])

        def qk_chunk(j, sec, c):
            """One n-chunk of q (sec=0) / k (sec=1) for head-pair j."""
            rhs = xT if sec == 0 else xRT
            dst = (qs if sec == 0 else ks)[j][:]
            ps = ring_tile(f"qk{sec}_{j}_{c}")
            for kt in range(CT):
                if j == 0:
                    wsl = wqk0t[:, kt * 256 + sec * 128:
                                kt * 256 + sec * 128 + 128]
                else:
                    wsl = wqk[kt][:, sec * C + j * 128:
                                  sec * C + j * 128 + 128]
                nc.tensor.matmul(ps[:], wsl,
                                 rhs[kt][:, c * 512:(c + 1) * 512],
                                 start=(kt == 0), stop=(kt == CT - 1))
            nc.vector.tensor_copy(dst[:, c * 512:(c + 1) * 512], ps[:])

        def qk_group(j, sec):
            qk_chunk(j, sec, 0)
            qk_chunk(j, sec, 1)

        def v_group(t):
            """v rows for (reversed) token tile t: [128 n, 768 o] -> vaug."""
            dst = vaug[t][:].rearrange("p (h u) -> p h u", u=65)[:, :, 0:64]
            dst = dst.rearrange("p (a g) d -> p a g d", a=2)
            for vc in range(2):
                ps = ring_tile(f"v{t}_{vc}")
                for kt in range(CT):
                    nc.tensor.matmul(ps[:, 0:384],
                                     xRT[kt][:, t * 128:(t + 1) * 128],
                                     wv[kt][:, vc * 384:(vc + 1) * 384],
                                     start=(kt == 0), stop=(kt == CT - 1))
                nc.vector.tensor_copy(
                    dst[:, vc],
                    ps[:, 0:384].rearrange("p (g d) -> p g d", d=64))

        qk_group(0, 0)
        qk_group(0, 1)

        # ---------------- attention ----------------
        # Cascaded AV deferral: pair j's AV matmuls run during pair j+1 (all
        # 16 P tiles of a pair stay resident in SBUF), so the per-pair PE work
        # in steady state is just scores + the previous pair's AV -- well
        # under the ACT (exp) pace that bounds each pair. The single 3-bank
        # accumulator still suffices: pair j's accumulation window is pair
        # j+1, released by the normalize at pair j+1's end.
        def finish_pair(pj, avf, phs):
            """Normalize + transpose for pair pj (AV already accumulated)."""
            rsb = rsp.tile([128, 18], F32, tag="rs", name=f"rs{pj}")
            den = avf.rearrange("p (g x) -> p g x", g=3)[:, :, 0:510]
            den = den.rearrange("p g (r y) -> p g r y", y=170)
            den = den.rearrange("p g r (h z) -> p g r h z", z=85)
            den = den[:, :, :, :, 64:65]
            rv = rsb[:].rearrange("p (g r h) -> p g r h", g=3, r=3)
            with nc.allow_low_precision(reason="softmax recip in f32"):
                nc.vector.reciprocal(rv.unsqueeze(-1), den)
            on = onp.tile([128, N], BF16, tag="on", name=f"on{pj}")
            for g in range(3):
                rc = 3 if g < 2 else 2
                src_v = avf[:, g * 512:g * 512 + 170 * rc].rearrange(
                    "p (r z) -> p r z", z=170)
                src_v = src_v.rearrange("p r (h y) -> p r h y",
                                        y=85)[:, :, :, 0:64]
                dst_v = on[:, g * 384:g * 384 + 128 * rc].rearrange(
                    "p (r h d) -> p r h d", h=2, d=64)
                sc_v = rsb[:, g * 6:g * 6 + 2 * rc].rearrange(
                    "p (r h) -> p r h", h=2).unsqueeze(-1)
                sc_v = sc_v.broadcast_to([128, rc, 2, 64])
                nc.vector.tensor_mul(dst_v, src_v, sc_v)
            # [q, c'] -> [c', q] through the DMA crossbar
            nc.sync.dma_start_transpose(
                outT[pj][:].rearrange("p (a b) -> p a b", b=128), on[:])

        def new_av(pj):
            av = avp.tile([128, 1536], F32, tag="av", name=f"av{pj}")
            avf = av[:]
            for g in range(3):
                nc.tensor.matmul(avf[:, g * 512:(g + 1) * 512], z1[:],
                                 z512[:], start=True, stop=True)
            return avf

        def av_batch(avf, pj, phs, hi, t):
            h = 2 * pj + hi
            ph = phs[hi][t]
            for qt in range(NT):
                so = _slot(hi, qt)
                nc.tensor.matmul(avf[:, so:so + 65],
                                 ph[:, qt * 128:(qt + 1) * 128],
                                 vaug[t][:, h * 65:h * 65 + 65],
                                 start=False, stop=(t == NT - 1),
                                 skip_group_check=True)

        prev_phs = None
        avf = None
        for j in range(H // 2):
            if j > 0:
                avf = new_av(j - 1)
            cur_phs = {0: [], 1: []}
            for t in range(NT):
                if t == 0 and j < 5:
                    tbls[2 * j + 2] = load_tbl(2 * j + 2)
                    tbls[2 * j + 3] = load_tbl(2 * j + 3)
                for hi in range(2):
                    h = 2 * j + hi
                    ee = eep.tile([128, N], BF16, tag="ee", name=f"ee{h}_{t}")
                    for c in range(2):
                        ps = ring_tile(f"sc{h}_{t}_{c}")
                        nc.tensor.matmul(
                            ps[:],
                            ks[j][hi * 64:hi * 64 + 64, t * 128:(t + 1) * 128],
                            qs[j][hi * 64:hi * 64 + 64, c * 512:(c + 1) * 512],
                            start=True, stop=True)
                        nc.scalar.activation(ee[:, c * 512:(c + 1) * 512],
                                             ps[:], EXP, scale=SCALE)
                    if j > 0:
                        av_batch(avf, j - 1, prev_phs, hi, t)
                    ph = php.tile([128, N], BF16, tag="ph", name=f"ph{h}_{t}")
                    tv = tbls[h][:, 252 * t:252 * t + 2016].rearrange(
                        "p (c a b) -> p c a b", c=2, b=63)[:, :, :, 0:32]
                    ev = ee[:].rearrange("p (c a b) -> p c a b", c=2, b=32)
                    pv = ph[:].rearrange("p (c a b) -> p c a b", c=2, b=32)
                    nc.vector.tensor_mul(pv, ev, tv)
                    cur_phs[hi].append(ph)
                # interleaved production for upcoming consumers
                if j == 0 and t < 6:
                    v_group(t)
                if j == 1 and t in (5, 6):
                    v_group(t + 1)
                if j < 5 and 3 <= t <= 6:
                    qk_chunk(j + 1, (t - 3) // 2, (t - 3) % 2)
            if j > 0:
                finish_pair(j - 1, avf, prev_phs)
            prev_phs = cur_phs

        # phantom pair: accumulate + finish pair 5 (proj groups for the
        # same t are interleaved so the PE chews their kt<4 steps while the
        # last transposes land)
        avf = new_av(5)

        for t in range(NT):
            for hi in range(2):
                av_batch(avf, 5, prev_phs, hi, t)
        finish_pair(5, avf, prev_phs)

        # ---------------- proj ----------------
        warm(14, "fill")
        for t in range(NT):
            f = fp.tile([128, C], F32, tag="f", name=f"f{t}")
            for pc in range(2):
                ps = ring_tile(f"pj{t}_{pc}")
                for kt in range(CT):
                    nc.tensor.matmul(ps[:, 0:384],
                                     outT[kt][:, t * 128:(t + 1) * 128],
                                     pw[kt][:, pc * 384:(pc + 1) * 384],
                                     start=(kt == 0), stop=(kt == CT - 1))
                nc.vector.tensor_add(f[:, pc * 384:(pc + 1) * 384],
                                     ps[:, 0:384],
                                     pbrow[:, pc * 384:(pc + 1) * 384])
            nc.sync.dma_start(out_d.ap()[t * 128:(t + 1) * 128, :], f[:])


_GRAPH = None


def _graph():
    global _GRAPH
    if _GRAPH is None:
        _GRAPH = _build_graph()
    return _GRAPH


def _host_prep(x, qkv_w, proj_w, proj_b, rpb_w1, rpb_b1, rpb_w2, rpb_b2):
    """Numpy layout prep + exp of the 63x63 bias table (7 MFLOP)."""
    import ml_dtypes
    a = np.arange(63, dtype=np.float32) - 31.0
    rel_y = np.broadcast_to(a[:, None], (63, 63))
    rel_x = np.broadcast_to(a[None, :], (63, 63))
    rel = np.stack([rel_x, rel_y], -1).reshape(-1, 2)           # [3969, 2]
    hdn = np.maximum(rel @ rpb_w1.T + rpb_b1, 0.0)
    gtbl = (hdn @ rpb_w2.T + rpb_b2).T.astype(np.float32)       # [12, 3969]
    gtbl = np.exp(gtbl, dtype=np.float32)                       # exp(bias)
    gpad = np.zeros((H, TW), np.float32)
    gpad[:, :3969] = gtbl
    gpad = gpad.astype(ml_dtypes.bfloat16)

    bf = ml_dtypes.bfloat16

    def fold(a):
        """[C, W] -> [128, CT*W]: channel tile kt becomes a free-dim block."""
        w = a.shape[1]
        return np.ascontiguousarray(
            a.reshape(CT, 128, w).transpose(1, 0, 2).reshape(128, CT * w))

    wqkvT = qkv_w.T.astype(bf)                                  # [768, 2304]
    wqkT = fold(wqkvT[:, 0:2 * C])
    wqk0 = np.ascontiguousarray(np.concatenate(
        [wqkvT[:, 0:128], wqkvT[:, C:C + 128]],
        axis=1).reshape(CT, 128, 256).transpose(1, 0, 2).reshape(128, -1))
    wvT = fold(wqkvT[:, 2 * C:3 * C])
    wprojT = fold(proj_w.T.astype(bf))                          # [768, 768]
    pbrep = np.ascontiguousarray(
        np.broadcast_to(proj_b.astype(np.float32), (128, C)))
    shared = {"qk_wT": wqkT, "qk0_wT": wqk0, "v_wT": wvT, "proj_wT": wprojT,
              "proj_b_rep": pbrep, "rpb_tbl": gpad}
    in_maps = []
    for i in range(B):
        m = dict(shared)
        m["xT"] = fold(x[i].T.astype(bf))
        m["xRT"] = fold(x[i][::-1].T.astype(bf))
        in_maps.append(m)
    return in_maps


def kernel(x, qkv_w, proj_w, proj_b, rpb_w1, rpb_b1, rpb_w2, rpb_b2,
           _trace=False, _tmpdir=None):
    in_maps = _host_prep(np.asarray(x), np.asarray(qkv_w), np.asarray(proj_w),
                         np.asarray(proj_b), np.asarray(rpb_w1),
                         np.asarray(rpb_b1), np.asarray(rpb_w2),
                         np.asarray(rpb_b2))
    nc = _graph()
    res = run_bass_kernel_spmd(nc, in_maps, core_ids=list(range(B)),
                               trace=_trace, tmpdir=_tmpdir)
    out = np.stack([res.results[i]["out"] for i in range(B)])
    if _trace:
        kernel._last_results = res
    return out


# revision 45
# speedup vs baseline: 1.2978x; 1.0040x over previous
"""Multi-head attention with relative-position-bias MLP on 8 TRN2 NeuronCores.

Strategy: pure data-parallel over batch (B=8 -> 1 batch element per core, no
collectives). All matmul operands are bf16 (fp8 was measured to fail the 2e-2
gate: softmax does not attenuate logit noise relative to output magnitude).
Host-side prep is layout only: per-core transposed x (plus a token-reversed
copy feeding k/v), transposed weights, replicated proj bias, and exp() of the
63x63 relative-position bias table.

Device algorithm per core (N=1024 tokens, C=768, H=12 heads, D=64):
  qT[o,n] = wqkv[:, o].T @ xT        (per head-pair o-block, bf16)
  kT[o,n] = wqkv[:, C+o].T @ xRT     (token-reversed k)
  v[n,o]  = xRT.T @ wqkv[:, 2C+o]    (token-reversed v, + ones column)
  per head h, k-tile t (128 reversed tokens):
     sT_h = kT_h(t).T @ qT_h          [nk=128, nq=1024]
     E = exp(sT * SCALE)              (ACT, PSUM->SBUF bf16)
     P = E * expB_tile                (DVE, all-SBUF bf16;
                                       exp(s+b) = exp(s)*exp(b))
     av[q, h-slot] += P(:, qtile).T @ [v_h(t) | 1]   (flipped AV: q on
                                       partitions, 65 free rows per matmul --
                                       half the PE rows of the [65, q] layout)
  normalize: recip of the ones-column sums, per-partition tensor_scalar mul
  (PSUM->SBUF bf16) -- no PE broadcast needed since the denominator is a
  per-q-partition scalar in this layout.
  transpose [q, c] -> [c, q] via the DMA crossbar (dma_start_transpose,
  16x128 xbar tiles; no PE or DVE cost) to feed proj's stationary side.
  final = outT.T @ projT (+ proj_b via a DVE add on the PSUM->SBUF copy)

Token reversal trick: bias[h,n,m] depends on grid coords of (n,m) only via
(cy_n - cy_m, cx_n - cx_m). Reversing key/value token order makes the
Toeplitz expansion all-positive-stride: TBLREP_h[p, J] = expG_h[63*(p//32)
+ p%32 + J] (4 plain DMAs per head), and each [128,1024] bias tile is a
strided view of it. The AV reduction over k-tiles is order-invariant.

Schedule: five single-bank PSUM ring tiles (a tile pool tag-ring, so Tile
pipelines producers against consumers) carry score chunks and the qk / v /
proj production groups; a 3-bank accumulator holds one head-pair's AV, with
all its 6-slots-per-bank cleared by one full-bank zero matmul per bank
(start=True zeroes the whole 2KB ZERO_REGION, so per-slot starts would erase
siblings). Cascaded AV deferral: pair j's 16 P tiles stay resident in SBUF
and its AV batches run during pair j+1, so the steady-state per-pair PE work
(scores + previous pair's AV) sits well under the ACT exp pace that bounds
each pair; normalize/transpose for pair j happen at pair j+1's end, and a
phantom stage after pair 5 finishes the last pair interleaved with proj.
v-production runs inside pairs 0-1; pair j+1's q/k production and bias-table
DMAs are interleaved into pair j; a dedicated first-pair weight slice plus
warm-up matmuls (PE p-state: 2x cycle cost for ~3us after idle) shorten the
DMA-bound startup. Loads: x via Pool SWDGE, weights/tables via SP HWDGE
(parallel issue paths; consolidated one-DMA-per-tensor layouts).
"""
import sys

import numpy as np

sys.path.insert(0, "/opt/trn_rl_repo")

import concourse.bass as bass  # noqa: E402
import concourse.mybir as mybir  # noqa: E402
import concourse.tile as tile  # noqa: E402
from concourse import bacc  # noqa: E402
from concourse.bass_utils import run_bass_kernel_spmd  # noqa: E402

F32 = mybir.dt.float32
BF16 = mybir.dt.bfloat16
EXP = mybir.ActivationFunctionType.Exp

B, N, C, H, D = 8, 1024, 768, 12, 64
SCALE = float(D) ** -0.5
NT = N // 128   # 8 token tiles
CT = C // 128   # 6 channel tiles
TBLW = 3781     # TBLREP width (padded so 2016-wide views stay in range)
TW = 4001       # DRAM table width per head (>= 220 + TBLW, zero-padded)


def _slot(hi, qt):
    """Free-offset (f32 elems) of head hi / q-tile qt in the AV accumulator.

    3 q-tiles per 2KB PSUM bank (170-elem stride), heads 85 apart, so every
    65-wide matmul output stays inside one bank.
    """
    return (qt // 3) * 512 + (qt % 3) * 170 + 85 * hi


def _rcol(hi, qt):
    return (qt // 3) * 6 + (qt % 3) * 2 + hi


def _build_graph():
    nc = bacc.Bacc("TRN2", target_bir_lowering=False, debug=False,
                   enable_asserts=False, num_devices=B)
    xT_d = nc.dram_tensor("xT", [128, CT * N], BF16, kind="ExternalInput")
    xRT_d = nc.dram_tensor("xRT", [128, CT * N], BF16,
                           kind="ExternalInput")
    wqk_d = nc.dram_tensor("qk_wT", [128, CT * 2 * C], BF16,
                           kind="ExternalInput")
    wqk0_d = nc.dram_tensor("qk0_wT", [128, CT * 256], BF16,
                            kind="ExternalInput")
    wv_d = nc.dram_tensor("v_wT", [128, CT * C], BF16, kind="ExternalInput")
    wproj_d = nc.dram_tensor("proj_wT", [128, CT * C], BF16,
                             kind="ExternalInput")
    pbrep_d = nc.dram_tensor("proj_b_rep", [128, C], F32, kind="ExternalInput")
    tbl_d = nc.dram_tensor("rpb_tbl", [H, TW], BF16, kind="ExternalInput")
    out_d = nc.dram_tensor("out", [N, C], F32, kind="ExternalOutput")

    with tile.TileContext(nc) as tc:
        _kern(tc, nc, xT_d, xRT_d, wqk_d, wqk0_d, wv_d, wproj_d, pbrep_d,
              tbl_d, out_d)
    nc.compile()
    return nc


def _kern(tc, nc, xT_d, xRT_d, wqk_d, wqk0_d, wv_d, wproj_d, pbrep_d,
          tbl_d, out_d):
    from contextlib import ExitStack

    with ExitStack() as es:
        persist = es.enter_context(tc.tile_pool(name="persist", bufs=1))
        xTt = persist.tile([128, CT * N], BF16, tag="xt", name="xt")
        xRTt = persist.tile([128, CT * N], BF16, tag="xr", name="xr")
        wqkt = persist.tile([128, CT * 2 * C], BF16, tag="w", name="w")
        wvt = persist.tile([128, CT * C], BF16, tag="wv", name="wv")
        pwt = persist.tile([128, CT * C], BF16, tag="pw", name="pw")
        wqk0t = persist.tile([128, CT * 256], BF16, tag="w0", name="w0")
        xT = [xTt[:, i * N:(i + 1) * N] for i in range(CT)]
        xRT = [xRTt[:, i * N:(i + 1) * N] for i in range(CT)]
        wqk = [wqkt[:, i * 2 * C:(i + 1) * 2 * C] for i in range(CT)]
        wv = [wvt[:, i * C:(i + 1) * C] for i in range(CT)]
        pw = [pwt[:, i * C:(i + 1) * C] for i in range(CT)]
        pbrow = persist.tile([128, C], F32, tag="pb", name="pbrow")
        qs = [persist.tile([128, N], BF16, tag=f"q{j}", name=f"q{j}")
              for j in range(H // 2)]
        ks = [persist.tile([128, N], BF16, tag=f"k{j}", name=f"k{j}")
              for j in range(H // 2)]
        vaug = [persist.tile([128, H * 65], BF16, tag=f"va{t}", name=f"va{t}")
                for t in range(NT)]
        outT = [persist.tile([128, N], BF16, tag=f"ot{i}", name=f"ot{i}")
                for i in range(CT)]

        tblp = es.enter_context(tc.tile_pool(name="tblp", bufs=4))
        eep = es.enter_context(tc.tile_pool(name="eep", bufs=4))
        php = es.enter_context(tc.tile_pool(name="php", bufs=19))
        onp = es.enter_context(tc.tile_pool(name="onp", bufs=2))
        rsp = es.enter_context(tc.tile_pool(name="rsp", bufs=2))
        fp = es.enter_context(tc.tile_pool(name="fp", bufs=3))
        # 5 single-bank PSUM tiles shared (tag-ring) by score chunks and the
        # qk / v / proj production groups: distinct tiles per allocation is
        # what lets Tile pipeline producers against consumers
        ringp = es.enter_context(
            tc.tile_pool(name="ring", bufs=5, space="PSUM"))
        avp = es.enter_context(tc.tile_pool(name="avp", bufs=1, space="PSUM"))

        def ring_tile(name):
            return ringp.tile([128, 512], F32, tag="rg", name=name, bufs=5)

        # ones column of the augmented-v tiles (denominator rides the AV
        # matmul's 65th free column)
        for t in range(NT):
            va = vaug[t][:].rearrange("p (h u) -> p h u", u=65)
            nc.gpsimd.memset(va[:, :, 64:65], 1.0)
        # zero operands for the AV-bank clearing matmuls: a start=True matmul
        # zeroes its whole 2KB PSUM bank (ZERO_REGION), so the 6 slots sharing
        # a bank must be cleared by one full-bank matmul, not per-slot starts
        z1 = persist.tile([1, 128], BF16, tag="z1", name="z1")
        nc.gpsimd.memset(z1[:], 0.0)
        z512 = persist.tile([1, 512], BF16, tag="z512", name="z512")
        nc.gpsimd.memset(z512[:], 0.0)

        # ---- loads: one consolidated DMA per tensor class; x on the Pool
        # SWDGE path, weights/tables on the SP HWDGE path (parallel issue).
        # While they are in flight, dummy matmuls keep the PE busy so its
        # p-state ramp (2x cycle cost for the first 3us after idle) is paid
        # on junk work instead of the first real groups. ----
        def warm(n, label):
            for wi in range(n):
                wt = ring_tile(f"{label}{wi}")
                nc.tensor.matmul(wt[:], z1[:], z512[:], start=True, stop=True)

        warm(30, "warm")
        nc.sync.dma_start(wqk0t[:], wqk0_d.ap()[:, :])
        nc.gpsimd.dma_start(xTt[:], xT_d.ap()[:, :])
        nc.gpsimd.dma_start(xRTt[:], xRT_d.ap()[:, :])

        def load_tbl(h):
            t = tblp.tile([128, TBLW], BF16, tag="tbl", name=f"tbl{h}")
            for blk in range(4):
                nc.sync.dma_start(
                    t[blk * 32:(blk + 1) * 32, :],
                    bass.AP(tbl_d, h * TW + 63 * blk, [[1, 32], [1, TBLW]]))
            return t

        tbls = {0: load_tbl(0)}
        nc.sync.dma_start(wvt[:], wv_d.ap()[:, :])
        tbls[1] = load_tbl(1)
        nc.sync.dma_start(wqkt[:], wqk_d.ap()[:, :])

        # proj weights late: they are only needed at the tail
        nc.sync.dma_start(pwt[:], wproj_d.ap()[:, :])
        nc.sync.dma_start(pbrow[:], pbrep_d.ap()[:, :])

        def qk_chunk(j, sec, c):
            """One n-chunk of q (sec=0) / k (sec=1) for head-pair j."""
            rhs = xT if sec == 0 else xRT
            dst = (qs if sec == 0 else ks)[j][:]
            ps = ring_tile(f"qk{sec}_{j}_{c}")
            for kt in range(CT):
                if j == 0:
                    wsl = wqk0t[:, kt * 256 + sec * 128:
                                kt * 256 + sec * 128 + 128]
                else:
                    wsl = wqk[kt][:, sec * C + j * 128:
                                  sec * C + j * 128 + 128]
                nc.tensor.matmul(ps[:], wsl,
                                 rhs[kt][:, c * 512:(c + 1) * 512],
                                 start=(kt == 0), stop=(kt == CT - 1))
            nc.vector.tensor_copy(dst[:, c * 512:(c + 1) * 512], ps[:])

        def qk_group(j, sec):
            qk_chunk(j, sec, 0)
            qk_chunk(j, sec, 1)

        def v_chunk(t, vc):
            """Heads vc*6..vc*6+5 of v for (reversed) token tile t -> vaug.
            The vc=0 half is first consumed by pair 0's deferred AV (heads
            0-1) during pair 1; the vc=1 half (heads 6-11) not before pair
            4 -- so one single-slot insert per iteration suffices."""
            dst = vaug[t][:].rearrange("p (h u) -> p h u", u=65)[:, :, 0:64]
            dst = dst.rearrange("p (a g) d -> p a g d", a=2)
            ps = ring_tile(f"v{t}_{vc}")
            for kt in range(CT):
                nc.tensor.matmul(ps[:, 0:384],
                                 xRT[kt][:, t * 128:(t + 1) * 128],
                                 wv[kt][:, vc * 384:(vc + 1) * 384],
                                 start=(kt == 0), stop=(kt == CT - 1))
            nc.vector.tensor_copy(
                dst[:, vc],
                ps[:, 0:384].rearrange("p (g d) -> p g d", d=64))

        qk_group(0, 0)
        qk_group(0, 1)

        # ---------------- attention ----------------
        # Cascaded AV deferral: pair j's AV matmuls run during pair j+1 (all
        # 16 P tiles of a pair stay resident in SBUF), so the per-pair PE work
        # in steady state is just scores + the previous pair's AV -- well
        # under the ACT (exp) pace that bounds each pair. The single 3-bank
        # accumulator still suffices: pair j's accumulation window is pair
        # j+1, released by the normalize at pair j+1's end.
        def finish_pair(pj, avf, phs):
            """Normalize + transpose for pair pj (AV already accumulated)."""
            rsb = rsp.tile([128, 18], F32, tag="rs", name=f"rs{pj}")
            den = avf.rearrange("p (g x) -> p g x", g=3)[:, :, 0:510]
            den = den.rearrange("p g (r y) -> p g r y", y=170)
            den = den.rearrange("p g r (h z) -> p g r h z", z=85)
            den = den[:, :, :, :, 64:65]
            rv = rsb[:].rearrange("p (g r h) -> p g r h", g=3, r=3)
            with nc.allow_low_precision(reason="softmax recip in f32"):
                nc.vector.reciprocal(rv.unsqueeze(-1), den)
            on = onp.tile([128, N], BF16, tag="on", name=f"on{pj}")
            for g in range(3):
                rc = 3 if g < 2 else 2
                src_v = avf[:, g * 512:g * 512 + 170 * rc].rearrange(
                    "p (r z) -> p r z", z=170)
                src_v = src_v.rearrange("p r (h y) -> p r h y",
                                        y=85)[:, :, :, 0:64]
                dst_v = on[:, g * 384:g * 384 + 128 * rc].rearrange(
                    "p (r h d) -> p r h d", h=2, d=64)
                sc_v = rsb[:, g * 6:g * 6 + 2 * rc].rearrange(
                    "p (r h) -> p r h", h=2).unsqueeze(-1)
                sc_v = sc_v.broadcast_to([128, rc, 2, 64])
                nc.vector.tensor_mul(dst_v, src_v, sc_v)
            # [q, c'] -> [c', q] through the DMA crossbar
            nc.sync.dma_start_transpose(
                outT[pj][:].rearrange("p (a b) -> p a b", b=128), on[:])

        def new_av(pj):
            av = avp.tile([128, 1536], F32, tag="av", name=f"av{pj}")
            avf = av[:]
            for g in range(3):
                nc.tensor.matmul(avf[:, g * 512:(g + 1) * 512], z1[:],
                                 z512[:], start=True, stop=True)
            return avf

        def av_batch(avf, pj, phs, hi, t):
            h = 2 * pj + hi
            ph = phs[hi][t]
            for qt in range(NT):
                so = _slot(hi, qt)
                nc.tensor.matmul(avf[:, so:so + 65],
                                 ph[:, qt * 128:(qt + 1) * 128],
                                 vaug[t][:, h * 65:h * 65 + 65],
                                 start=False, stop=(t == NT - 1),
                                 skip_group_check=True)

        prev_phs = None
        avf = None
        for j in range(H // 2):
            if j > 0:
                avf = new_av(j - 1)
            cur_phs = {0: [], 1: []}
            for t in range(NT):
                if t == 0 and j < 5:
                    tbls[2 * j + 2] = load_tbl(2 * j + 2)
                    tbls[2 * j + 3] = load_tbl(2 * j + 3)
                for hi in range(2):
                    h = 2 * j + hi
                    ee = eep.tile([128, N], BF16, tag="ee", name=f"ee{h}_{t}")
                    for c in range(2):
                        ps = ring_tile(f"sc{h}_{t}_{c}")
                        nc.tensor.matmul(
                            ps[:],
                            ks[j][hi * 64:hi * 64 + 64, t * 128:(t + 1) * 128],
                            qs[j][hi * 64:hi * 64 + 64, c * 512:(c + 1) * 512],
                            start=True, stop=True)
                        nc.scalar.activation(ee[:, c * 512:(c + 1) * 512],
                                             ps[:], EXP, scale=SCALE)
                    if j > 0:
                        av_batch(avf, j - 1, prev_phs, hi, t)
                    ph = php.tile([128, N], BF16, tag="ph", name=f"ph{h}_{t}")
                    tv = tbls[h][:, 252 * t:252 * t + 2016].rearrange(
                        "p (c a b) -> p c a b", c=2, b=63)[:, :, :, 0:32]
                    ev = ee[:].rearrange("p (c a b) -> p c a b", c=2, b=32)
                    pv = ph[:].rearrange("p (c a b) -> p c a b", c=2, b=32)
                    nc.vector.tensor_mul(pv, ev, tv)
                    cur_phs[hi].append(ph)
                # interleaved production for upcoming consumers
                if j == 0:
                    v_chunk(t, 0)
                if j == 1:
                    v_chunk(t, 1)
                if j < 5 and 3 <= t <= 6:
                    qk_chunk(j + 1, (t - 3) // 2, (t - 3) % 2)
            if j > 0:
                finish_pair(j - 1, avf, prev_phs)
            prev_phs = cur_phs

        # phantom pair: accumulate + finish pair 5 (proj groups for the
        # same t are interleaved so the PE chews their kt<4 steps while the
        # last transposes land)
        avf = new_av(5)

        for t in range(NT):
            for hi in range(2):
                av_batch(avf, 5, prev_phs, hi, t)
        finish_pair(5, avf, prev_phs)

        # ---------------- proj ----------------
        warm(14, "fill")
        for t in range(NT):
            f = fp.tile([128, C], F32, tag="f", name=f"f{t}")
            for pc in range(2):
                ps = ring_tile(f"pj{t}_{pc}")
                for kt in range(CT):
                    nc.tensor.matmul(ps[:, 0:384],
                                     outT[kt][:, t * 128:(t + 1) * 128],
                                     pw[kt][:, pc * 384:(pc + 1) * 384],
                                     start=(kt == 0), stop=(kt == CT - 1))
                nc.vector.tensor_add(f[:, pc * 384:(pc + 1) * 384],
                                     ps[:, 0:384],
                                     pbrow[:, pc * 384:(pc + 1) * 384])
            nc.sync.dma_start(out_d.ap()[t * 128:(t + 1) * 128, :], f[:])


_GRAPH = None


def _graph():
    global _GRAPH
    if _GRAPH is None:
        _GRAPH = _build_graph()
    return _GRAPH


def _host_prep(x, qkv_w, proj_w, proj_b, rpb_w1, rpb_b1, rpb_w2, rpb_b2):
    """Numpy layout prep + exp of the 63x63 bias table (7 MFLOP)."""
    import ml_dtypes
    a = np.arange(63, dtype=np.float32) - 31.0
    rel_y = np.broadcast_to(a[:, None], (63, 63))
    rel_x = np.broadcast_to(a[None, :], (63, 63))
    rel = np.stack([rel_x, rel_y], -1).reshape(-1, 2)           # [3969, 2]
    hdn = np.maximum(rel @ rpb_w1.T + rpb_b1, 0.0)
    gtbl = (hdn @ rpb_w2.T + rpb_b2).T.astype(np.float32)       # [12, 3969]
    gtbl = np.exp(gtbl, dtype=np.float32)                       # exp(bias)
    gpad = np.zeros((H, TW), np.float32)
    gpad[:, :3969] = gtbl
    gpad = gpad.astype(ml_dtypes.bfloat16)

    bf = ml_dtypes.bfloat16

    def fold(a):
        """[C, W] -> [128, CT*W]: channel tile kt becomes a free-dim block."""
        w = a.shape[1]
        return np.ascontiguousarray(
            a.reshape(CT, 128, w).transpose(1, 0, 2).reshape(128, CT * w))

    wqkvT = qkv_w.T.astype(bf)                                  # [768, 2304]
    wqkT = fold(wqkvT[:, 0:2 * C])
    wqk0 = np.ascontiguousarray(np.concatenate(
        [wqkvT[:, 0:128], wqkvT[:, C:C + 128]],
        axis=1).reshape(CT, 128, 256).transpose(1, 0, 2).reshape(128, -1))
    wvT = fold(wqkvT[:, 2 * C:3 * C])
    wprojT = fold(proj_w.T.astype(bf))                          # [768, 768]
    pbrep = np.ascontiguousarray(
        np.broadcast_to(proj_b.astype(np.float32), (128, C)))
    shared = {"qk_wT": wqkT, "qk0_wT": wqk0, "v_wT": wvT, "proj_wT": wprojT,
              "proj_b_rep": pbrep, "rpb_tbl": gpad}
    in_maps = []
    for i in range(B):
        m = dict(shared)
        m["xT"] = fold(x[i].T.astype(bf))
        m["xRT"] = fold(x[i][::-1].T.astype(bf))
        in_maps.append(m)
    return in_maps


def kernel(x, qkv_w, proj_w, proj_b, rpb_w1, rpb_b1, rpb_w2, rpb_b2,
           _trace=False, _tmpdir=None):
    in_maps = _host_prep(np.asarray(x), np.asarray(qkv_w), np.asarray(proj_w),
                         np.asarray(proj_b), np.asarray(rpb_w1),
                         np.asarray(rpb_b1), np.asarray(rpb_w2),
                         np.asarray(rpb_b2))
    nc = _graph()
    res = run_bass_kernel_spmd(nc, in_maps, core_ids=list(range(B)),
                               trace=_trace, tmpdir=_tmpdir)
    out = np.stack([res.results[i]["out"] for i in range(B)])
    if _trace:
        kernel._last_results = res
    return out
